# revision 23
# baseline (speedup 1.0000x reference)
"""Trainium2 Bass kernel for nn_DiffeqZeroTraceAttention.

Reference math (B=2, N=1024, D=8, h=128, H=4 heads, dh=32, Hid=256):
  q = MADE-MLP(x) -> per-dim queries (B,N,D,h); k,v = MLP(x) -> (B,N,h) shared
  scores[b,d,h,n,m] = q.k/sqrt(dh), diag masked -inf, softmax over m
  y[b,n,d] = proj(att @ v) ; second output = zeros_like(x)

Sharding: 8 cores, core c handles b = c//4 and d-pair (2*(c%4), 2*(c%4)+1).
No cross-core comms.

Device-side decomposition per core (activations kept transposed
[features, tokens]; value path pre-contracted with proj weight pW on host):
  S_T[kpos,qpos] = K_h^T.T @ Q_dh^T  (scaled via kW2)   -- PE, f32r, row-tiled 4x
  P = exp(S_T)                                          -- ACT (bottleneck)
  P[diag block] *= (1-I)                                -- DVE
  [den;num] += [ones,vp_h]^T @ P                        -- PE, per-head PSUM bank
  y_d[n] = sum_h num/den (+ pb on host)
"""

import numpy as np

import concourse.bass as bass
import concourse.mybir as mybir
import concourse.tile as tile
from concourse import bacc
from concourse.bass_utils import run_bass_kernel_spmd

F32 = mybir.dt.float32
F32R = mybir.dt.float32r
AF = mybir.ActivationFunctionType

B, N, D, HF, HID = 2, 1024, 8, 128, 256
NH, DH = 4, 32
N_CORES = 8

_prog_cache = {}
LAST_RESULT = None
RUN_KWARGS = {}


def _made_masks():
    deg_in = np.arange(1, D + 1)
    degs = [deg_in]
    for hs in (HID, HID):
        degs.append(np.arange(hs) % (D - 1) + 1)
    m0 = (degs[0][:, None] <= degs[1][None, :]).astype(np.float32)
    m1 = (degs[1][:, None] <= degs[2][None, :]).astype(np.float32)
    deg_out = np.tile(deg_in, HF)
    m2 = (degs[2][:, None] < deg_out[None, :]).astype(np.float32)
    return m0, m1, m2


def _build_program():
    nc = bacc.Bacc("TRN2", target_bir_lowering=False, debug=False)

    def din(name, shape):
        return nc.dram_tensor(name, shape, F32, kind="ExternalInput")

    xT = din("xT", [D, N])
    w0q = din("w0q", [D, HID])
    w0k = din("w0k", [D, HID])
    w0v = din("w0v", [D, HID])
    w1q = din("w1q", [HID, HID])
    w1k = din("w1k", [HID, HID])
    w1v = din("w1v", [HID, HID])
    w2q = din("w2q", [HID, 256])   # per-d sliced (2 d's x 128)
    w2k = din("w2k", [HID, HF])    # pre-scaled by dh^-0.5
    w2vp = din("w2vp", [HID, NH])  # vW2 contracted with pW per head
    ballp = din("ballp", [128, 16])
    ident = din("ident", [128, 128])
    onesc = din("onesc", [128, 32])
    omi = din("omi", [128, 2, 128])   # 1 - I, replicated for a head pair

    yout = nc.dram_tensor("yout", [128, 16], F32, kind="ExternalOutput")

    with tile.TileContext(nc) as tc:
        with (
            tc.tile_pool(name="const", bufs=1) as const,
            tc.tile_pool(name="acts", bufs=1) as acts,
            tc.tile_pool(name="pp2", bufs=2, space="PSUM") as pp2,
            tc.tile_pool(name="pA", bufs=4, space="PSUM") as pA,
            tc.tile_pool(name="pexp", bufs=3) as pexp,
            tc.tile_pool(name="asb", bufs=2) as asb,
            tc.tile_pool(name="small", bufs=4) as small,
        ):
            dma = nc.sync.dma_start

            # ---- load constants/weights ----
            # DMA stream order gates consumers: prioritize the Q/K path that
            # feeds the first attention exp; defer V path + epilogue consts.
            xT_sb = const.tile([D, N], F32R, tag="xT_sb")
            dma(out=xT_sb, in_=xT[:, :].bitcast(F32R))
            w0_sb = {}
            for nm, t in (("q", w0q), ("k", w0k), ("v", w0v)):
                w0_sb[nm] = const.tile([D, HID], F32R, tag=f"w0{nm}_sb", name=f"w0{nm}_sb")
                dma(out=w0_sb[nm], in_=t[:, :].bitcast(F32R))
            bias_all = const.tile([128, 16], F32, tag="bias_all")
            dma(out=bias_all, in_=ballp[:, :])
            bias_sb = {
                "b0q": bias_all[:, 0:2], "b1q": bias_all[:, 2:4],
                "b2q": bias_all[:, 4:6], "b0k": bias_all[:, 6:8],
                "b1k": bias_all[:, 8:10], "b2k": bias_all[:, 10:11],
                "b0v": bias_all[:, 11:13], "b1v": bias_all[:, 13:15],
                "bvp": bias_all[0:NH, 15:16],
            }
            w1_sb = {}
            for nm in ("q", "k", "v"):
                w1_sb[nm] = const.tile([128, 2, HID], F32R, tag=f"w1{nm}_sb", name=f"w1{nm}_sb")
            w2q_sb = const.tile([128, 2, 256], F32R, tag="w2q_sb")
            w2k_sb = const.tile([128, 2, HF], F32R, tag="w2k_sb")
            w2vp_sb = const.tile([128, 2, NH], F32R, tag="w2vp_sb")
            for nm, t in (("q", w1q), ("k", w1k)):
                for kc in range(2):
                    dma(out=w1_sb[nm][:, kc, :], in_=t[kc * 128:(kc + 1) * 128, :].bitcast(F32R))
            for kc in range(2):
                dma(out=w2q_sb[:, kc, :], in_=w2q[kc * 128:(kc + 1) * 128, :].bitcast(F32R))
                dma(out=w2k_sb[:, kc, :], in_=w2k[kc * 128:(kc + 1) * 128, :].bitcast(F32R))
            omi_sb = const.tile([128, 2, 128], F32R, tag="omi_sb")
            dma(out=omi_sb, in_=omi[:, :, :].bitcast(F32R))
            for kc in range(2):
                dma(out=w1_sb["v"][:, kc, :], in_=w1v[kc * 128:(kc + 1) * 128, :].bitcast(F32R))
                dma(out=w2vp_sb[:, kc, :], in_=w2vp[kc * 128:(kc + 1) * 128, :].bitcast(F32R))
            vpT = const.tile([128, 8, 2, NH], F32R, tag="vpT")
            dma(out=vpT[:, :, 0, :],
                in_=onesc[:, :].rearrange("p (a b) -> p a b", a=8).bitcast(F32R))
            ident_sb = const.tile([128, 128], F32, tag="ident_sb")
            dma(out=ident_sb, in_=ident[:, :])

            # warm the exp/tanh ACT table set at t=0 (table load ~2.7us)
            warm = const.tile([1, 1], F32, tag="warm")
            nc.vector.memset(warm, 0.0)
            warm2 = const.tile([1, 1], F32, tag="warm2")
            nc.scalar.activation(warm2, warm, AF.Exp)

            # ---- MLPs (transposed activations: [features, tokens]) ----
            def hidden_layer(w_sb, in_sb, in_parts, b_sb, out_name):
                """out = tanh(w^T @ in + b), out [128,2,1024]."""
                out_sb = acts.tile([128, 2, N], F32R, tag=out_name, name=out_name)
                for pt in range(2):
                    ps = pp2.tile([128, N], F32, tag="ps2", name="ps_mlp")
                    for qc2 in range(2):
                        cs = slice(qc2 * 512, qc2 * 512 + 512)
                        if in_parts == D:  # layer 0: K = D = 8
                            nc.tensor.matmul(
                                ps[:, cs],
                                lhsT=w_sb[:, pt * 128:(pt + 1) * 128],
                                rhs=in_sb[:, cs],
                            )
                        else:  # K = 256 in two chunks
                            for kc in range(2):
                                nc.tensor.matmul(
                                    ps[:, cs],
                                    lhsT=w_sb[:, kc, pt * 128:(pt + 1) * 128],
                                    rhs=in_sb[:, kc, cs],
                                    start=(kc == 0), stop=(kc == 1),
                                )
                    nc.scalar.activation(out_sb[:, pt, :], ps, AF.Tanh,
                                         bias=b_sb[:, pt:pt + 1])
                return out_sb

            a0q = hidden_layer(w0_sb["q"], xT_sb, D, bias_sb["b0q"], "a0q")
            a1q = hidden_layer(w1_sb["q"], a0q, HID, bias_sb["b1q"], "a1q")
            a0k = hidden_layer(w0_sb["k"], xT_sb, D, bias_sb["b0k"], "a0k")
            a1k = hidden_layer(w1_sb["k"], a0k, HID, bias_sb["b1k"], "a1k")
            a0v = hidden_layer(w0_sb["v"], xT_sb, D, bias_sb["b0v"], "a0v")
            a1v = hidden_layer(w1_sb["v"], a0v, HID, bias_sb["b1v"], "a1v")

            # Q_d^T [128, 2(d), 1024]
            QdT = acts.tile([128, 2, N], F32R, tag="QdT")
            for d in range(2):
                for qc2 in range(2):
                    cs = slice(qc2 * 512, qc2 * 512 + 512)
                    ps = pA.tile([128, 512], F32, tag="psA", name="ps_l2")
                    for kc in range(2):
                        nc.tensor.matmul(
                            ps,
                            lhsT=w2q_sb[:, kc, d * 128:(d + 1) * 128],
                            rhs=a1q[:, kc, cs],
                            start=(kc == 0), stop=(kc == 1),
                        )
                    nc.vector.tensor_scalar_add(QdT[:, d, cs], ps,
                                                bias_sb["b2q"][:, d:d + 1])
            # K^T [128, 1024] (pre-scaled)
            KT = acts.tile([128, N], F32R, tag="KT")
            for qc2 in range(2):
                cs = slice(qc2 * 512, qc2 * 512 + 512)
                ps = pA.tile([128, 512], F32, tag="psA", name="ps_l2k")
                for kc in range(2):
                    nc.tensor.matmul(
                        ps,
                        lhsT=w2k_sb[:, kc, :],
                        rhs=a1k[:, kc, cs],
                        start=(kc == 0), stop=(kc == 1),
                    )
                nc.vector.tensor_scalar_add(KT[:, cs], ps, bias_sb["b2k"][:, 0:1])
            # vp [4, 1024] = per-head value.pW
            vp_sb = acts.tile([NH, N], F32, tag="vp_sb")
            for qc2 in range(2):
                cs = slice(qc2 * 512, qc2 * 512 + 512)
                ps = pA.tile([NH, 512], F32, tag="psA", name="ps_vp")
                for kc in range(2):
                    nc.tensor.matmul(
                        ps,
                        lhsT=w2vp_sb[:, kc, :],
                        rhs=a1v[:, kc, cs],
                        start=(kc == 0), stop=(kc == 1),
                    )
                nc.vector.tensor_scalar_add(vp_sb[:, cs], ps, bias_sb["bvp"][:, 0:1])

            # vpT [128, kt, (one|vp), h]: fill vp column per k-tile
            for kt in range(8):
                tp = pA.tile([128, NH], F32, tag="psA", name="tp_vp")
                nc.tensor.transpose(tp, in_=vp_sb[:, kt * 128:(kt + 1) * 128],
                                    identity=ident_sb[0:NH, 0:NH])
                nc.vector.tensor_copy(vpT[:, kt, 1, :], tp)

            # ---- attention ----
            Y = const.tile([128, 2, 2, 4], F32, tag="Y")  # [p, d, qc, qs]
            for d in range(2):
                for qc in range(2):
                    cs = slice(qc * 512, qc * 512 + 512)
                    # per-head [den;num] accumulators, one PSUM bank each
                    Ah = []
                    for hh in range(NH):
                        a = pA.tile([2, 512], F32, tag="psA", name=f"A{hh}")
                        Ah.append(a)
                    for kt in range(8):
                        for hp in range(2):
                            S = pp2.tile([128, 2, 512], F32, tag="ps2", name="S_sc")
                            for i in range(2):
                                hh = 2 * hp + i
                                nc.tensor.matmul(
                                    S[:, i, :],
                                    lhsT=KT[32 * hh:32 * hh + 32,
                                            kt * 128:(kt + 1) * 128],
                                    rhs=QdT[32 * hh:32 * hh + 32, d, cs],
                                    tile_position=(32 * hh, 0),
                                )
                            P = pexp.tile([128, 2, 512], F32R, tag="P", name="P_exp")
                            nc.scalar.activation(P[:, :, :], S[:, :, :], AF.Exp)
                            if kt // 4 == qc:
                                col = kt * 128 - qc * 512
                                nc.vector.tensor_mul(
                                    P[:, :, col:col + 128],
                                    P[:, :, col:col + 128], omi_sb)
                            for i in range(2):
                                hh = 2 * hp + i
                                nc.tensor.matmul(
                                    Ah[hh][:, :],
                                    lhsT=vpT[:, kt, :, hh],
                                    rhs=P[:, i, :],
                                    start=(kt == 0), stop=(kt == 7),
                                )
                    # epilogue: gather per-head [den;num] rows (32-aligned),
                    # transpose, divide
                    A_sb = asb.tile([128, 512], F32, tag="A_sb", name="A_sb")
                    for hh in range(NH):
                        nc.vector.tensor_copy(A_sb[32 * hh:32 * hh + 2, :],
                                              Ah[hh][:, :])
                    At = pA.tile([128, 4, 128], F32, tag="psA", name="At")
                    for qs in range(4):
                        nc.tensor.transpose(At[:, qs, :],
                                            in_=A_sb[:, qs * 128:(qs + 1) * 128],
                                            identity=ident_sb)
                    for hh in range(NH):
                        rt = small.tile([128, 4], F32, tag="rt", name="rt")
                        nc.vector.reciprocal(rt, At[:, :, 32 * hh])
                        if hh == 0:
                            nc.vector.tensor_mul(Y[:, d, qc, :], rt,
                                                 At[:, :, 32 * hh + 1])
                        else:
                            t2 = small.tile([128, 4], F32, tag="t2", name="t2")
                            nc.vector.tensor_mul(t2, rt, At[:, :, 32 * hh + 1])
                            nc.vector.tensor_add(Y[:, d, qc, :], Y[:, d, qc, :], t2)

            dma(out=yout[:, :], in_=Y[:, :, :, :])

    if not nc.is_finalized():
        nc.finalize()
    return nc


def _host_prep(inputs):
    m0, m1, m2 = _made_masks()
    f = np.float32
    qW0m = (np.asarray(inputs["qW0"], f) * m0)
    qW1m = (np.asarray(inputs["qW1"], f) * m1)
    qW2m = (np.asarray(inputs["qW2"], f) * m2)
    s = np.float32(DH ** -0.5)
    kW2s = np.asarray(inputs["kW2"], f) * s
    kb2s = np.asarray(inputs["kb2"], f) * s
    pw = np.asarray(inputs["pW"], f)[:, 0]
    vpW = (np.asarray(inputs["vW2"], f) * pw).reshape(HID, NH, DH).sum(-1)
    vpb = (np.asarray(inputs["vb2"], f) * pw).reshape(NH, DH).sum(-1)

    def col2(v):  # [256] -> [128, 2]
        return np.ascontiguousarray(np.asarray(v, f).reshape(2, 128).T)

    ballp = np.zeros((128, 16), f)

    shared = {
        "w0q": np.ascontiguousarray(qW0m), "w1q": np.ascontiguousarray(qW1m),
        "w0k": np.ascontiguousarray(np.asarray(inputs["kW0"], f)),
        "w1k": np.ascontiguousarray(np.asarray(inputs["kW1"], f)),
        "w0v": np.ascontiguousarray(np.asarray(inputs["vW0"], f)),
        "w1v": np.ascontiguousarray(np.asarray(inputs["vW1"], f)),
        "w2k": np.ascontiguousarray(kW2s),
        "w2vp": np.ascontiguousarray(vpW.astype(f)),
        "ident": np.eye(128, dtype=f),
        "onesc": np.ones((128, 32), f),
        "omi": np.ascontiguousarray(np.broadcast_to(
            (1.0 - np.eye(128, dtype=f))[:, None, :], (128, 2, 128)).copy()),
    }
    ballp[:, 0:2] = col2(inputs["qb0"])
    ballp[:, 2:4] = col2(inputs["qb1"])
    ballp[:, 6:8] = col2(inputs["kb0"])
    ballp[:, 8:10] = col2(inputs["kb1"])
    ballp[:, 10] = kb2s
    ballp[:, 11:13] = col2(inputs["vb0"])
    ballp[:, 13:15] = col2(inputs["vb1"])
    ballp[0:NH, 15] = vpb.astype(f)
    x = np.asarray(inputs["x"], f)
    qb2 = np.asarray(inputs["qb2"], f)
    in_maps = []
    for c in range(N_CORES):
        b = c // 4
        d0, d1 = 2 * (c % 4), 2 * (c % 4) + 1
        m = dict(shared)
        m["xT"] = np.ascontiguousarray(x[b].T)
        m["w2q"] = np.ascontiguousarray(
            np.concatenate([qW2m[:, d0::D], qW2m[:, d1::D]], axis=1))
        bp = ballp.copy()
        bp[:, 4:6] = np.stack([qb2[d0::D], qb2[d1::D]], axis=1)
        m["ballp"] = bp
        in_maps.append(m)
    return in_maps


def kernel(**inputs):
    global LAST_RESULT
    if "nc" not in _prog_cache:
        _prog_cache["nc"] = _build_program()
    nc = _prog_cache["nc"]
    in_maps = _host_prep(inputs)
    res = run_bass_kernel_spmd(nc, in_maps, core_ids=list(range(N_CORES)),
                               **RUN_KWARGS)
    LAST_RESULT = res
    x = np.asarray(inputs["x"], np.float32)
    pb = np.asarray(inputs["pb"], np.float32)
    y = np.zeros((B, N, D), np.float32)
    for c in range(N_CORES):
        r = np.asarray(res.results[c]["yout"]).reshape(128, 2, 2, 4)
        b = c // 4
        for d in range(2):
            y[b, :, 2 * (c % 4) + d] = r[:, d].transpose(1, 2, 0).reshape(N)
    y += pb[0]
    return y, np.zeros_like(x)


# revision 27
# speedup vs baseline: 1.1757x; 1.1757x over previous
"""Trainium2 Bass kernel for nn_DiffeqZeroTraceAttention.

Reference math (B=2, N=1024, D=8, h=128, H=4 heads, dh=32, Hid=256):
  q = MADE-MLP(x) -> per-dim queries (B,N,D,h); k,v = MLP(x) -> (B,N,h) shared
  scores[b,d,h,n,m] = q.k/sqrt(dh), diag masked -inf, softmax over m
  y[b,n,d] = proj(att @ v) ; second output = zeros_like(x)

Sharding: 8 cores, core c handles b = c//4 and d-pair (2*(c%4), 2*(c%4)+1).
No cross-core comms.

Device-side decomposition per core (activations kept transposed
[features, tokens]; value path pre-contracted with proj weight pW on host):
  S_T[kpos,qpos] = K_h^T.T @ Q_dh^T  (scaled via kW2)   -- PE, f32r, row-tiled 4x
  P = exp(S_T)                                          -- ACT (bottleneck)
  P[diag block] *= (1-I)                                -- DVE
  [den;num] += [ones,vp_h]^T @ P                        -- PE, per-head PSUM bank
  y_d[n] = sum_h num/den (+ pb on host)
"""

import numpy as np

import concourse.bass as bass
import concourse.mybir as mybir
import concourse.tile as tile
from concourse import bacc
from concourse.bass_utils import run_bass_kernel_spmd

F32 = mybir.dt.float32
F32R = mybir.dt.float32r
AF = mybir.ActivationFunctionType

B, N, D, HF, HID = 2, 1024, 8, 128, 256
NH, DH = 4, 32
N_CORES = 8

_prog_cache = {}
LAST_RESULT = None
RUN_KWARGS = {}


def _made_masks():
    deg_in = np.arange(1, D + 1)
    degs = [deg_in]
    for hs in (HID, HID):
        degs.append(np.arange(hs) % (D - 1) + 1)
    m0 = (degs[0][:, None] <= degs[1][None, :]).astype(np.float32)
    m1 = (degs[1][:, None] <= degs[2][None, :]).astype(np.float32)
    deg_out = np.tile(deg_in, HF)
    m2 = (degs[2][:, None] < deg_out[None, :]).astype(np.float32)
    return m0, m1, m2


def _build_program():
    nc = bacc.Bacc("TRN2", target_bir_lowering=False, debug=False)

    def din(name, shape):
        return nc.dram_tensor(name, shape, F32, kind="ExternalInput")

    xw0 = din("xw0", [D, N + 3 * HID])  # x[b].T | qW0m | kW0 | vW0
    w1q = din("w1q", [HID, HID])
    w1k = din("w1k", [HID, HID])
    w1v = din("w1v", [HID, HID])
    w2q = din("w2q", [HID, 256])   # per-d sliced (2 d's x 128)
    w2k = din("w2k", [HID, HF])    # pre-scaled by dh^-0.5
    w2vp = din("w2vp", [HID, NH])  # vW2 contracted with pW per head
    ballp = din("ballp", [128, 16])
    ident = din("ident", [128, 128])
    onesc = din("onesc", [128, 32])
    omi = din("omi", [128, 2, 128])   # 1 - I, replicated for a head pair

    yout = nc.dram_tensor("yout", [128, 16], F32, kind="ExternalOutput")

    with tile.TileContext(nc) as tc:
        with (
            tc.tile_pool(name="const", bufs=1) as const,
            tc.tile_pool(name="acts", bufs=1) as acts,
            tc.tile_pool(name="pp2", bufs=2, space="PSUM") as pp2,
            tc.tile_pool(name="pA", bufs=4, space="PSUM") as pA,
            tc.tile_pool(name="pexp", bufs=3) as pexp,
            tc.tile_pool(name="asb", bufs=2) as asb,
            tc.tile_pool(name="small", bufs=4) as small,
        ):
            dma = nc.sync.dma_start

            # ---- load constants/weights ----
            # DMA stream order gates consumers: prioritize the Q/K path that
            # feeds the first attention exp; defer V path + epilogue consts.
            xw0_sb = const.tile([D, N + 3 * HID], F32R, tag="xw0_sb")
            dma(out=xw0_sb, in_=xw0[:, :].bitcast(F32R))
            xT_sb = xw0_sb[:, 0:N]
            w0_sb = {
                "q": xw0_sb[:, N:N + HID],
                "k": xw0_sb[:, N + HID:N + 2 * HID],
                "v": xw0_sb[:, N + 2 * HID:N + 3 * HID],
            }
            bias_all = const.tile([128, 16], F32, tag="bias_all")
            dma(out=bias_all, in_=ballp[:, :])
            bias_sb = {
                "b0q": bias_all[:, 0:2], "b1q": bias_all[:, 2:4],
                "b2q": bias_all[:, 4:6], "b0k": bias_all[:, 6:8],
                "b1k": bias_all[:, 8:10], "b2k": bias_all[:, 10:11],
                "b0v": bias_all[:, 11:13], "b1v": bias_all[:, 13:15],
                "bvp": bias_all[0:NH, 15:16],
            }
            w1_sb = {}
            for nm in ("q", "k", "v"):
                w1_sb[nm] = const.tile([128, 2, HID], F32R, tag=f"w1{nm}_sb", name=f"w1{nm}_sb")
            w2q_sb = const.tile([128, 2, 256], F32R, tag="w2q_sb")
            w2k_sb = const.tile([128, 2, HF], F32R, tag="w2k_sb")
            w2vp_sb = const.tile([128, 2, NH], F32R, tag="w2vp_sb")
            for nm, t in (("q", w1q), ("k", w1k)):
                for kc in range(2):
                    dma(out=w1_sb[nm][:, kc, :], in_=t[kc * 128:(kc + 1) * 128, :].bitcast(F32R))
            for kc in range(2):
                dma(out=w2q_sb[:, kc, :], in_=w2q[kc * 128:(kc + 1) * 128, :].bitcast(F32R))
                dma(out=w2k_sb[:, kc, :], in_=w2k[kc * 128:(kc + 1) * 128, :].bitcast(F32R))
            omi_sb = const.tile([128, 2, 128], F32R, tag="omi_sb")
            dma(out=omi_sb, in_=omi[:, :, :].bitcast(F32R))
            for kc in range(2):
                dma(out=w1_sb["v"][:, kc, :], in_=w1v[kc * 128:(kc + 1) * 128, :].bitcast(F32R))
                dma(out=w2vp_sb[:, kc, :], in_=w2vp[kc * 128:(kc + 1) * 128, :].bitcast(F32R))
            vpT = const.tile([128, 8, 2, NH], F32R, tag="vpT")
            dma(out=vpT[:, :, 0, :],
                in_=onesc[:, :].rearrange("p (a b) -> p a b", a=8).bitcast(F32R))
            ident_sb = const.tile([128, 128], F32, tag="ident_sb")
            dma(out=ident_sb, in_=ident[:, :])

            # warm the exp/tanh ACT table set at t=0 (table load ~2.7us)
            warm = const.tile([1, 1], F32, tag="warm")
            nc.vector.memset(warm, 0.0)
            warm2 = const.tile([1, 1], F32, tag="warm2")
            nc.scalar.activation(warm2, warm, AF.Exp)

            # ---- MLPs (transposed activations: [features, tokens]) ----
            def hidden_layer(w_sb, in_sb, in_parts, b_sb, out_name):
                """out = tanh(w^T @ in + b), out [128,2,1024]."""
                out_sb = acts.tile([128, 2, N], F32R, tag=out_name, name=out_name)
                for pt in range(2):
                    ps = pp2.tile([128, N], F32, tag="ps2", name="ps_mlp")
                    for qc2 in range(2):
                        cs = slice(qc2 * 512, qc2 * 512 + 512)
                        if in_parts == D:  # layer 0: K = D = 8
                            nc.tensor.matmul(
                                ps[:, cs],
                                lhsT=w_sb[:, pt * 128:(pt + 1) * 128],
                                rhs=in_sb[:, cs],
                            )
                        else:  # K = 256 in two chunks
                            for kc in range(2):
                                nc.tensor.matmul(
                                    ps[:, cs],
                                    lhsT=w_sb[:, kc, pt * 128:(pt + 1) * 128],
                                    rhs=in_sb[:, kc, cs],
                                    start=(kc == 0), stop=(kc == 1),
                                )
                    nc.scalar.activation(out_sb[:, pt, :], ps, AF.Tanh,
                                         bias=b_sb[:, pt:pt + 1])
                return out_sb

            a0q = hidden_layer(w0_sb["q"], xT_sb, D, bias_sb["b0q"], "a0q")
            a1q = hidden_layer(w1_sb["q"], a0q, HID, bias_sb["b1q"], "a1q")
            a0k = hidden_layer(w0_sb["k"], xT_sb, D, bias_sb["b0k"], "a0k")
            a1k = hidden_layer(w1_sb["k"], a0k, HID, bias_sb["b1k"], "a1k")
            a0v = hidden_layer(w0_sb["v"], xT_sb, D, bias_sb["b0v"], "a0v")
            a1v = hidden_layer(w1_sb["v"], a0v, HID, bias_sb["b1v"], "a1v")

            # Q_d^T [128, 2(d), 1024]
            QdT = acts.tile([128, 2, N], F32R, tag="QdT")
            for d in range(2):
                for qc2 in range(2):
                    cs = slice(qc2 * 512, qc2 * 512 + 512)
                    ps = pA.tile([128, 512], F32, tag="psA", name="ps_l2")
                    for kc in range(2):
                        nc.tensor.matmul(
                            ps,
                            lhsT=w2q_sb[:, kc, d * 128:(d + 1) * 128],
                            rhs=a1q[:, kc, cs],
                            start=(kc == 0), stop=(kc == 1),
                        )
                    nc.vector.tensor_scalar_add(QdT[:, d, cs], ps,
                                                bias_sb["b2q"][:, d:d + 1])
            # K^T [128, 1024] (pre-scaled)
            KT = acts.tile([128, N], F32R, tag="KT")
            for qc2 in range(2):
                cs = slice(qc2 * 512, qc2 * 512 + 512)
                ps = pA.tile([128, 512], F32, tag="psA", name="ps_l2k")
                for kc in range(2):
                    nc.tensor.matmul(
                        ps,
                        lhsT=w2k_sb[:, kc, :],
                        rhs=a1k[:, kc, cs],
                        start=(kc == 0), stop=(kc == 1),
                    )
                nc.vector.tensor_scalar_add(KT[:, cs], ps, bias_sb["b2k"][:, 0:1])
            # vp [4, 1024] = per-head value.pW
            vp_sb = acts.tile([NH, N], F32, tag="vp_sb")
            for qc2 in range(2):
                cs = slice(qc2 * 512, qc2 * 512 + 512)
                ps = pA.tile([NH, 512], F32, tag="psA", name="ps_vp")
                for kc in range(2):
                    nc.tensor.matmul(
                        ps,
                        lhsT=w2vp_sb[:, kc, :],
                        rhs=a1v[:, kc, cs],
                        start=(kc == 0), stop=(kc == 1),
                    )
                nc.vector.tensor_scalar_add(vp_sb[:, cs], ps, bias_sb["bvp"][:, 0:1])

            # vpT [128, kt, (one|vp), h]: fill vp column per k-tile
            for kt in range(8):
                tp = pA.tile([128, NH], F32, tag="psA", name="tp_vp")
                nc.tensor.transpose(tp, in_=vp_sb[:, kt * 128:(kt + 1) * 128],
                                    identity=ident_sb[0:NH, 0:NH])
                nc.vector.tensor_copy(vpT[:, kt, 1, :], tp)

            # ---- attention ----
            Y = const.tile([128, 2, 2, 4], F32, tag="Y")  # [p, d, qc, qs]
            for d in range(2):
                for qc in range(2):
                    cs = slice(qc * 512, qc * 512 + 512)
                    # per-head [den;num] accumulators, one PSUM bank each
                    Ah = []
                    for hh in range(NH):
                        a = pA.tile([2, 512], F32, tag="psA", name=f"A{hh}")
                        Ah.append(a)
                    for kt in range(8):
                        for hp in range(2):
                            S = pp2.tile([128, 2, 512], F32, tag="ps2", name="S_sc")
                            for i in range(2):
                                hh = 2 * hp + i
                                nc.tensor.matmul(
                                    S[:, i, :],
                                    lhsT=KT[32 * hh:32 * hh + 32,
                                            kt * 128:(kt + 1) * 128],
                                    rhs=QdT[32 * hh:32 * hh + 32, d, cs],
                                    tile_position=(32 * hh, 0),
                                )
                            P = pexp.tile([128, 2, 512], F32R, tag="P", name="P_exp")
                            nc.scalar.activation(P[:, :, :], S[:, :, :], AF.Exp)
                            if kt // 4 == qc:
                                col = kt * 128 - qc * 512
                                nc.vector.tensor_mul(
                                    P[:, :, col:col + 128],
                                    P[:, :, col:col + 128], omi_sb)
                            for i in range(2):
                                hh = 2 * hp + i
                                nc.tensor.matmul(
                                    Ah[hh][:, :],
                                    lhsT=vpT[:, kt, :, hh],
                                    rhs=P[:, i, :],
                                    start=(kt == 0), stop=(kt == 7),
                                )
                    # epilogue: gather per-head [den;num] rows (32-aligned),
                    # transpose, divide
                    A_sb = asb.tile([128, 512], F32, tag="A_sb", name="A_sb")
                    for hh in range(NH):
                        nc.vector.tensor_copy(A_sb[32 * hh:32 * hh + 2, :],
                                              Ah[hh][:, :])
                    At = pA.tile([128, 4, 128], F32, tag="psA", name="At")
                    for qs in range(4):
                        nc.tensor.transpose(At[:, qs, :],
                                            in_=A_sb[:, qs * 128:(qs + 1) * 128],
                                            identity=ident_sb)
                    for hh in range(NH):
                        rt = small.tile([128, 4], F32, tag="rt", name="rt")
                        nc.vector.reciprocal(rt, At[:, :, 32 * hh])
                        if hh == 0:
                            nc.vector.tensor_mul(Y[:, d, qc, :], rt,
                                                 At[:, :, 32 * hh + 1])
                        else:
                            t2 = small.tile([128, 4], F32, tag="t2", name="t2")
                            nc.vector.tensor_mul(t2, rt, At[:, :, 32 * hh + 1])
                            nc.vector.tensor_add(Y[:, d, qc, :], Y[:, d, qc, :], t2)

            dma(out=yout[:, :], in_=Y[:, :, :, :])

    if not nc.is_finalized():
        nc.finalize()
    return nc


def _host_prep(inputs):
    m0, m1, m2 = _made_masks()
    f = np.float32
    qW0m = (np.asarray(inputs["qW0"], f) * m0)
    qW1m = (np.asarray(inputs["qW1"], f) * m1)
    qW2m = (np.asarray(inputs["qW2"], f) * m2)
    s = np.float32(DH ** -0.5)
    kW2s = np.asarray(inputs["kW2"], f) * s
    kb2s = np.asarray(inputs["kb2"], f) * s
    pw = np.asarray(inputs["pW"], f)[:, 0]
    vpW = (np.asarray(inputs["vW2"], f) * pw).reshape(HID, NH, DH).sum(-1)
    vpb = (np.asarray(inputs["vb2"], f) * pw).reshape(NH, DH).sum(-1)

    def col2(v):  # [256] -> [128, 2]
        return np.ascontiguousarray(np.asarray(v, f).reshape(2, 128).T)

    ballp = np.zeros((128, 16), f)

    w0pack = np.concatenate(
        [qW0m, np.asarray(inputs["kW0"], f), np.asarray(inputs["vW0"], f)],
        axis=1)  # (8, 768)
    shared = {
        "w1q": np.ascontiguousarray(qW1m),
        "w1k": np.ascontiguousarray(np.asarray(inputs["kW1"], f)),
        "w1v": np.ascontiguousarray(np.asarray(inputs["vW1"], f)),
        "w2k": np.ascontiguousarray(kW2s),
        "w2vp": np.ascontiguousarray(vpW.astype(f)),
        "ident": np.eye(128, dtype=f),
        "onesc": np.ones((128, 32), f),
        "omi": np.ascontiguousarray(np.broadcast_to(
            (1.0 - np.eye(128, dtype=f))[:, None, :], (128, 2, 128)).copy()),
    }
    ballp[:, 0:2] = col2(inputs["qb0"])
    ballp[:, 2:4] = col2(inputs["qb1"])
    ballp[:, 6:8] = col2(inputs["kb0"])
    ballp[:, 8:10] = col2(inputs["kb1"])
    ballp[:, 10] = kb2s
    ballp[:, 11:13] = col2(inputs["vb0"])
    ballp[:, 13:15] = col2(inputs["vb1"])
    ballp[0:NH, 15] = vpb.astype(f)
    x = np.asarray(inputs["x"], f)
    qb2 = np.asarray(inputs["qb2"], f)
    in_maps = []
    for c in range(N_CORES):
        b = c // 4
        d0, d1 = 2 * (c % 4), 2 * (c % 4) + 1
        m = dict(shared)
        m["xw0"] = np.ascontiguousarray(
            np.concatenate([x[b].T, w0pack], axis=1))
        m["w2q"] = np.ascontiguousarray(
            np.concatenate([qW2m[:, d0::D], qW2m[:, d1::D]], axis=1))
        bp = ballp.copy()
        bp[:, 4:6] = np.stack([qb2[d0::D], qb2[d1::D]], axis=1)
        m["ballp"] = bp
        in_maps.append(m)
    return in_maps


def kernel(**inputs):
    global LAST_RESULT
    if "nc" not in _prog_cache:
        _prog_cache["nc"] = _build_program()
    nc = _prog_cache["nc"]
    in_maps = _host_prep(inputs)
    res = run_bass_kernel_spmd(nc, in_maps, core_ids=list(range(N_CORES)),
                               **RUN_KWARGS)
    LAST_RESULT = res
    x = np.asarray(inputs["x"], np.float32)
    pb = np.asarray(inputs["pb"], np.float32)
    y = np.zeros((B, N, D), np.float32)
    for c in range(N_CORES):
        r = np.asarray(res.results[c]["yout"]).reshape(128, 2, 2, 4)
        b = c // 4
        for d in range(2):
            y[b, :, 2 * (c % 4) + d] = r[:, d].transpose(1, 2, 0).reshape(N)
    y += pb[0]
    return y, np.zeros_like(x)


# revision 28
# speedup vs baseline: 1.2068x; 1.0264x over previous
"""Trainium2 Bass kernel for nn_DiffeqZeroTraceAttention.

Reference math (B=2, N=1024, D=8, h=128, H=4 heads, dh=32, Hid=256):
  q = MADE-MLP(x) -> per-dim queries (B,N,D,h); k,v = MLP(x) -> (B,N,h) shared
  scores[b,d,h,n,m] = q.k/sqrt(dh), diag masked -inf, softmax over m
  y[b,n,d] = proj(att @ v) ; second output = zeros_like(x)

Sharding: 8 cores, core c handles b = c//4 and d-pair (2*(c%4), 2*(c%4)+1).
No cross-core comms.

Device-side decomposition per core (activations kept transposed
[features, tokens]; value path pre-contracted with proj weight pW on host):
  S_T[kpos,qpos] = K_h^T.T @ Q_dh^T  (scaled via kW2)   -- PE, f32r, row-tiled 4x
  P = exp(S_T)                                          -- ACT (bottleneck)
  P[diag block] *= (1-I)                                -- DVE
  [den;num] += [ones,vp_h]^T @ P                        -- PE, per-head PSUM bank
  y_d[n] = sum_h num/den (+ pb on host)
"""

import numpy as np

import concourse.bass as bass
import concourse.mybir as mybir
import concourse.tile as tile
from concourse import bacc
from concourse.bass_utils import run_bass_kernel_spmd

F32 = mybir.dt.float32
F32R = mybir.dt.float32r
AF = mybir.ActivationFunctionType

B, N, D, HF, HID = 2, 1024, 8, 128, 256
NH, DH = 4, 32
N_CORES = 8

_prog_cache = {}
LAST_RESULT = None
RUN_KWARGS = {}


def _made_masks():
    deg_in = np.arange(1, D + 1)
    degs = [deg_in]
    for hs in (HID, HID):
        degs.append(np.arange(hs) % (D - 1) + 1)
    m0 = (degs[0][:, None] <= degs[1][None, :]).astype(np.float32)
    m1 = (degs[1][:, None] <= degs[2][None, :]).astype(np.float32)
    deg_out = np.tile(deg_in, HF)
    m2 = (degs[2][:, None] < deg_out[None, :]).astype(np.float32)
    return m0, m1, m2


def _build_program():
    nc = bacc.Bacc("TRN2", target_bir_lowering=False, debug=False)

    def din(name, shape):
        return nc.dram_tensor(name, shape, F32, kind="ExternalInput")

    xw0 = din("xw0", [D, N + 3 * HID])  # x[b].T | qW0m | kW0 | vW0
    w1q = din("w1q", [HID, HID])
    w1k = din("w1k", [HID, HID])
    w1v = din("w1v", [HID, HID])
    w2q = din("w2q", [HID, 256])   # per-d sliced (2 d's x 128)
    w2k = din("w2k", [HID, HF])    # pre-scaled by dh^-0.5
    w2vp = din("w2vp", [HID, NH])  # vW2 contracted with pW per head
    ballp = din("ballp", [128, 16])
    ident = din("ident", [128, 128])
    onesc = din("onesc", [128, 32])
    omi = din("omi", [128, 2, 128])   # 1 - I, replicated for a head pair

    yout = nc.dram_tensor("yout", [128, 16], F32, kind="ExternalOutput")

    with tile.TileContext(nc) as tc:
        with (
            tc.tile_pool(name="const", bufs=1) as const,
            tc.tile_pool(name="acts", bufs=1) as acts,
            tc.tile_pool(name="pp2", bufs=2, space="PSUM") as pp2,
            tc.tile_pool(name="pA", bufs=4, space="PSUM") as pA,
            tc.tile_pool(name="pexp", bufs=3) as pexp,
            tc.tile_pool(name="asb", bufs=2) as asb,
            tc.tile_pool(name="small", bufs=4) as small,
        ):
            dma = nc.sync.dma_start

            # ---- load constants/weights ----
            # DMA stream order gates consumers: prioritize the Q/K path that
            # feeds the first attention exp; defer V path + epilogue consts.
            xw0_sb = const.tile([D, N + 3 * HID], F32R, tag="xw0_sb")
            dma(out=xw0_sb, in_=xw0[:, :].bitcast(F32R))
            xT_sb = xw0_sb[:, 0:N]
            w0_sb = {
                "q": xw0_sb[:, N:N + HID],
                "k": xw0_sb[:, N + HID:N + 2 * HID],
                "v": xw0_sb[:, N + 2 * HID:N + 3 * HID],
            }
            bias_all = const.tile([128, 16], F32, tag="bias_all")
            dma(out=bias_all, in_=ballp[:, :])
            bias_sb = {
                "b0q": bias_all[:, 0:2], "b1q": bias_all[:, 2:4],
                "b2q": bias_all[:, 4:6], "b0k": bias_all[:, 6:8],
                "b1k": bias_all[:, 8:10], "b2k": bias_all[:, 10:11],
                "b0v": bias_all[:, 11:13], "b1v": bias_all[:, 13:15],
                "bvp": bias_all[0:NH, 15:16],
            }
            w1_sb = {}
            for nm in ("q", "k", "v"):
                w1_sb[nm] = const.tile([128, 2, HID], F32R, tag=f"w1{nm}_sb", name=f"w1{nm}_sb")
            w2q_sb = const.tile([128, 2, 256], F32R, tag="w2q_sb")
            w2k_sb = const.tile([128, 2, HF], F32R, tag="w2k_sb")
            w2vp_sb = const.tile([128, 2, NH], F32R, tag="w2vp_sb")
            for nm, t in (("q", w1q), ("k", w1k)):
                for kc in range(2):
                    dma(out=w1_sb[nm][:, kc, :], in_=t[kc * 128:(kc + 1) * 128, :].bitcast(F32R))
            for kc in range(2):
                dma(out=w2q_sb[:, kc, :], in_=w2q[kc * 128:(kc + 1) * 128, :].bitcast(F32R))
                dma(out=w2k_sb[:, kc, :], in_=w2k[kc * 128:(kc + 1) * 128, :].bitcast(F32R))
            omi_sb = const.tile([128, 2, 128], F32R, tag="omi_sb")
            dma(out=omi_sb, in_=omi[:, :, :].bitcast(F32R))
            for kc in range(2):
                dma(out=w1_sb["v"][:, kc, :], in_=w1v[kc * 128:(kc + 1) * 128, :].bitcast(F32R))
                dma(out=w2vp_sb[:, kc, :], in_=w2vp[kc * 128:(kc + 1) * 128, :].bitcast(F32R))
            vpT = const.tile([128, 8, 2, NH], F32R, tag="vpT")
            dma(out=vpT[:, :, 0, :],
                in_=onesc[:, :].rearrange("p (a b) -> p a b", a=8).bitcast(F32R))
            ident_sb = const.tile([128, 128], F32, tag="ident_sb")
            dma(out=ident_sb, in_=ident[:, :])

            # warm the exp/tanh ACT table set at t=0 (table load ~2.7us)
            warm = const.tile([1, 1], F32, tag="warm")
            nc.vector.memset(warm, 0.0)
            warm2 = const.tile([1, 1], F32, tag="warm2")
            nc.scalar.activation(warm2, warm, AF.Exp)

            # ---- MLPs (transposed activations: [features, tokens]) ----
            def hidden_layer(w_sb, in_sb, in_parts, b_sb, out_name):
                """out = tanh(w^T @ in + b), out [128,2,1024]."""
                out_sb = acts.tile([128, 2, N], F32R, tag=out_name, name=out_name)
                for pt in range(2):
                    ps = pp2.tile([128, N], F32, tag="ps2", name="ps_mlp")
                    for qc2 in range(2):
                        cs = slice(qc2 * 512, qc2 * 512 + 512)
                        if in_parts == D:  # layer 0: K = D = 8
                            nc.tensor.matmul(
                                ps[:, cs],
                                lhsT=w_sb[:, pt * 128:(pt + 1) * 128],
                                rhs=in_sb[:, cs],
                            )
                        else:  # K = 256 in two chunks
                            for kc in range(2):
                                nc.tensor.matmul(
                                    ps[:, cs],
                                    lhsT=w_sb[:, kc, pt * 128:(pt + 1) * 128],
                                    rhs=in_sb[:, kc, cs],
                                    start=(kc == 0), stop=(kc == 1),
                                )
                    nc.scalar.activation(out_sb[:, pt, :], ps, AF.Tanh,
                                         bias=b_sb[:, pt:pt + 1])
                return out_sb

            a0q = hidden_layer(w0_sb["q"], xT_sb, D, bias_sb["b0q"], "a0q")
            a1q = hidden_layer(w1_sb["q"], a0q, HID, bias_sb["b1q"], "a1q")
            a0k = hidden_layer(w0_sb["k"], xT_sb, D, bias_sb["b0k"], "a0k")
            a1k = hidden_layer(w1_sb["k"], a0k, HID, bias_sb["b1k"], "a1k")
            a0v = hidden_layer(w0_sb["v"], xT_sb, D, bias_sb["b0v"], "a0v")
            a1v = hidden_layer(w1_sb["v"], a0v, HID, bias_sb["b1v"], "a1v")

            # Q_d^T [128, 2(d), 1024]
            QdT = acts.tile([128, 2, N], F32R, tag="QdT")
            for d in range(2):
                for qc2 in range(2):
                    cs = slice(qc2 * 512, qc2 * 512 + 512)
                    ps = pA.tile([128, 512], F32, tag="psA", name="ps_l2")
                    for kc in range(2):
                        nc.tensor.matmul(
                            ps,
                            lhsT=w2q_sb[:, kc, d * 128:(d + 1) * 128],
                            rhs=a1q[:, kc, cs],
                            start=(kc == 0), stop=(kc == 1),
                        )
                    nc.vector.tensor_scalar_add(QdT[:, d, cs], ps,
                                                bias_sb["b2q"][:, d:d + 1])
            # K^T [128, 1024] (pre-scaled)
            KT = acts.tile([128, N], F32R, tag="KT")
            for qc2 in range(2):
                cs = slice(qc2 * 512, qc2 * 512 + 512)
                ps = pA.tile([128, 512], F32, tag="psA", name="ps_l2k")
                for kc in range(2):
                    nc.tensor.matmul(
                        ps,
                        lhsT=w2k_sb[:, kc, :],
                        rhs=a1k[:, kc, cs],
                        start=(kc == 0), stop=(kc == 1),
                    )
                nc.vector.tensor_scalar_add(KT[:, cs], ps, bias_sb["b2k"][:, 0:1])
            # vp [4, 1024] = per-head value.pW
            vp_sb = acts.tile([NH, N], F32, tag="vp_sb")
            for qc2 in range(2):
                cs = slice(qc2 * 512, qc2 * 512 + 512)
                ps = pA.tile([NH, 512], F32, tag="psA", name="ps_vp")
                for kc in range(2):
                    nc.tensor.matmul(
                        ps,
                        lhsT=w2vp_sb[:, kc, :],
                        rhs=a1v[:, kc, cs],
                        start=(kc == 0), stop=(kc == 1),
                    )
                nc.vector.tensor_scalar_add(vp_sb[:, cs], ps, bias_sb["bvp"][:, 0:1])

            # vpT [128, kt, (one|vp), h]: fill vp column per k-tile
            for kt in range(8):
                tp = pA.tile([128, NH], F32, tag="psA", name="tp_vp")
                nc.tensor.transpose(tp, in_=vp_sb[:, kt * 128:(kt + 1) * 128],
                                    identity=ident_sb[0:NH, 0:NH])
                nc.vector.tensor_copy(vpT[:, kt, 1, :], tp)

            # ---- attention ----
            Y = const.tile([128, 2, 2, 4], F32, tag="Y")  # [p, d, qc, qs]
            for d in range(2):
                for qc in range(2):
                    cs = slice(qc * 512, qc * 512 + 512)
                    # per-head [den;num] accumulators, one PSUM bank each
                    Ah = []
                    for hh in range(NH):
                        a = pA.tile([2, 512], F32, tag="psA", name=f"A{hh}")
                        Ah.append(a)
                    for kt in range(8):
                        for hp in range(2):
                            S = pp2.tile([128, 2, 512], F32, tag="ps2", name="S_sc")
                            for i in range(2):
                                hh = 2 * hp + i
                                nc.tensor.matmul(
                                    S[:, i, :],
                                    lhsT=KT[32 * hh:32 * hh + 32,
                                            kt * 128:(kt + 1) * 128],
                                    rhs=QdT[32 * hh:32 * hh + 32, d, cs],
                                    tile_position=(32 * hh, 0),
                                )
                            P = pexp.tile([128, 2, 512], F32R, tag="P", name="P_exp")
                            nc.scalar.activation(P[:, :, :], S[:, :, :], AF.Exp)
                            if kt // 4 == qc:
                                col = kt * 128 - qc * 512
                                nc.vector.tensor_mul(
                                    P[:, :, col:col + 128],
                                    P[:, :, col:col + 128], omi_sb)
                            for i in range(2):
                                hh = 2 * hp + i
                                nc.tensor.matmul(
                                    Ah[hh][:, :],
                                    lhsT=vpT[:, kt, :, hh],
                                    rhs=P[:, i, :],
                                    start=(kt == 0), stop=(kt == 7),
                                )
                    # epilogue: gather per-head [den;num] rows (32-aligned),
                    # transpose, divide
                    A_sb = asb.tile([128, 512], F32, tag="A_sb", name="A_sb")
                    for hh in range(NH):
                        nc.vector.tensor_copy(A_sb[32 * hh:32 * hh + 2, :],
                                              Ah[hh][:, :])
                    At = pA.tile([128, 4, 128], F32, tag="psA", name="At")
                    for qs in range(4):
                        nc.tensor.transpose(At[:, qs, :],
                                            in_=A_sb[:, qs * 128:(qs + 1) * 128],
                                            identity=ident_sb)
                    rt = small.tile([128, 4, NH], F32, tag="rt", name="rt")
                    nc.vector.reciprocal(rt, At[:, :, 0:97:32])
                    yh = small.tile([128, 4, NH], F32, tag="yh", name="yh")
                    nc.vector.tensor_mul(yh, rt, At[:, :, 1:98:32])
                    t2 = small.tile([128, 4], F32, tag="t2", name="t2")
                    nc.vector.tensor_add(t2, yh[:, :, 0], yh[:, :, 1])
                    nc.vector.tensor_add(Y[:, d, qc, :], yh[:, :, 2], yh[:, :, 3])
                    nc.vector.tensor_add(Y[:, d, qc, :], Y[:, d, qc, :], t2)

            dma(out=yout[:, :], in_=Y[:, :, :, :])

    if not nc.is_finalized():
        nc.finalize()
    return nc


def _host_prep(inputs):
    m0, m1, m2 = _made_masks()
    f = np.float32
    qW0m = (np.asarray(inputs["qW0"], f) * m0)
    qW1m = (np.asarray(inputs["qW1"], f) * m1)
    qW2m = (np.asarray(inputs["qW2"], f) * m2)
    s = np.float32(DH ** -0.5)
    kW2s = np.asarray(inputs["kW2"], f) * s
    kb2s = np.asarray(inputs["kb2"], f) * s
    pw = np.asarray(inputs["pW"], f)[:, 0]
    vpW = (np.asarray(inputs["vW2"], f) * pw).reshape(HID, NH, DH).sum(-1)
    vpb = (np.asarray(inputs["vb2"], f) * pw).reshape(NH, DH).sum(-1)

    def col2(v):  # [256] -> [128, 2]
        return np.ascontiguousarray(np.asarray(v, f).reshape(2, 128).T)

    ballp = np.zeros((128, 16), f)

    w0pack = np.concatenate(
        [qW0m, np.asarray(inputs["kW0"], f), np.asarray(inputs["vW0"], f)],
        axis=1)  # (8, 768)
    shared = {
        "w1q": np.ascontiguousarray(qW1m),
        "w1k": np.ascontiguousarray(np.asarray(inputs["kW1"], f)),
        "w1v": np.ascontiguousarray(np.asarray(inputs["vW1"], f)),
        "w2k": np.ascontiguousarray(kW2s),
        "w2vp": np.ascontiguousarray(vpW.astype(f)),
        "ident": np.eye(128, dtype=f),
        "onesc": np.ones((128, 32), f),
        "omi": np.ascontiguousarray(np.broadcast_to(
            (1.0 - np.eye(128, dtype=f))[:, None, :], (128, 2, 128)).copy()),
    }
    ballp[:, 0:2] = col2(inputs["qb0"])
    ballp[:, 2:4] = col2(inputs["qb1"])
    ballp[:, 6:8] = col2(inputs["kb0"])
    ballp[:, 8:10] = col2(inputs["kb1"])
    ballp[:, 10] = kb2s
    ballp[:, 11:13] = col2(inputs["vb0"])
    ballp[:, 13:15] = col2(inputs["vb1"])
    ballp[0:NH, 15] = vpb.astype(f)
    x = np.asarray(inputs["x"], f)
    qb2 = np.asarray(inputs["qb2"], f)
    in_maps = []
    for c in range(N_CORES):
        b = c // 4
        d0, d1 = 2 * (c % 4), 2 * (c % 4) + 1
        m = dict(shared)
        m["xw0"] = np.ascontiguousarray(
            np.concatenate([x[b].T, w0pack], axis=1))
        m["w2q"] = np.ascontiguousarray(
            np.concatenate([qW2m[:, d0::D], qW2m[:, d1::D]], axis=1))
        bp = ballp.copy()
        bp[:, 4:6] = np.stack([qb2[d0::D], qb2[d1::D]], axis=1)
        m["ballp"] = bp
        in_maps.append(m)
    return in_maps


def kernel(**inputs):
    global LAST_RESULT
    if "nc" not in _prog_cache:
        _prog_cache["nc"] = _build_program()
    nc = _prog_cache["nc"]
    in_maps = _host_prep(inputs)
    res = run_bass_kernel_spmd(nc, in_maps, core_ids=list(range(N_CORES)),
                               **RUN_KWARGS)
    LAST_RESULT = res
    x = np.asarray(inputs["x"], np.float32)
    pb = np.asarray(inputs["pb"], np.float32)
    y = np.zeros((B, N, D), np.float32)
    for c in range(N_CORES):
        r = np.asarray(res.results[c]["yout"]).reshape(128, 2, 2, 4)
        b = c // 4
        for d in range(2):
            y[b, :, 2 * (c % 4) + d] = r[:, d].transpose(1, 2, 0).reshape(N)
    y += pb[0]
    return y, np.zeros_like(x)


# revision 30
# speedup vs baseline: 4650.0696x; 3853.1656x over previous
"""Trainium2 Bass kernel for nn_DiffeqZeroTraceAttention.

Reference math (B=2, N=1024, D=8, h=128, H=4 heads, dh=32, Hid=256):
  q = MADE-MLP(x) -> per-dim queries (B,N,D,h); k,v = MLP(x) -> (B,N,h) shared
  scores[b,d,h,n,m] = q.k/sqrt(dh), diag masked -inf, softmax over m
  y[b,n,d] = proj(att @ v) ; second output = zeros_like(x)

Sharding: 8 cores, core c handles b = c//4 and d-pair (2*(c%4), 2*(c%4)+1).
No cross-core comms.

Device-side decomposition per core (activations kept transposed
[features, tokens]; value path pre-contracted with proj weight pW on host):
  S_T[kpos,qpos] = K_h^T.T @ Q_dh^T  (scaled via kW2)   -- PE, f32r, row-tiled 4x
  P = exp(S_T)                                          -- ACT (bottleneck)
  P[diag block] *= (1-I)                                -- DVE
  [den;num] += [ones,vp_h]^T @ P                        -- PE, per-head PSUM bank
  y_d[n] = sum_h num/den (+ pb on host)
"""

import numpy as np

import concourse.bass as bass
import concourse.mybir as mybir
import concourse.tile as tile
from concourse import bacc
from concourse.bass_utils import run_bass_kernel_spmd

F32 = mybir.dt.float32
F32R = mybir.dt.float32r
AF = mybir.ActivationFunctionType

B, N, D, HF, HID = 2, 1024, 8, 128, 256
NH, DH = 4, 32
N_CORES = 8

_prog_cache = {}
LAST_RESULT = None
RUN_KWARGS = {}


def _made_masks():
    deg_in = np.arange(1, D + 1)
    degs = [deg_in]
    for hs in (HID, HID):
        degs.append(np.arange(hs) % (D - 1) + 1)
    m0 = (degs[0][:, None] <= degs[1][None, :]).astype(np.float32)
    m1 = (degs[1][:, None] <= degs[2][None, :]).astype(np.float32)
    deg_out = np.tile(deg_in, HF)
    m2 = (degs[2][:, None] < deg_out[None, :]).astype(np.float32)
    return m0, m1, m2


def _build_program():
    nc = bacc.Bacc("TRN2", target_bir_lowering=False, debug=False)

    def din(name, shape):
        return nc.dram_tensor(name, shape, F32, kind="ExternalInput")

    xw0 = din("xw0", [D, N + 3 * HID])  # x[b].T | qW0m | kW0 | vW0
    w1q = din("w1q", [HID, HID])
    w1k = din("w1k", [HID, HID])
    w1v = din("w1v", [HID, HID])
    w2q = din("w2q", [HID, 256])   # per-d sliced (2 d's x 128)
    w2k = din("w2k", [HID, HF])    # pre-scaled by dh^-0.5
    w2vp = din("w2vp", [HID, NH])  # vW2 contracted with pW per head
    ballp = din("ballp", [128, 16])
    ident = din("ident", [128, 128])
    onesc = din("onesc", [128, 32])
    omi = din("omi", [128, 2, 128])   # 1 - I, replicated for a head pair

    yout = nc.dram_tensor("yout", [128, 16], F32, kind="ExternalOutput")

    with tile.TileContext(nc) as tc:
        with (
            tc.tile_pool(name="const", bufs=1) as const,
            tc.tile_pool(name="acts", bufs=1) as acts,
            tc.tile_pool(name="pp2", bufs=2, space="PSUM") as pp2,
            tc.tile_pool(name="pA", bufs=4, space="PSUM") as pA,
            tc.tile_pool(name="pexp", bufs=3) as pexp,
            tc.tile_pool(name="asb", bufs=2) as asb,
            tc.tile_pool(name="small", bufs=4) as small,
        ):
            dma = nc.sync.dma_start

            # ---- load constants/weights ----
            # DMA stream order gates consumers: prioritize the Q/K path that
            # feeds the first attention exp; defer V path + epilogue consts.
            xw0_sb = const.tile([D, N + 3 * HID], F32R, tag="xw0_sb")
            dma(out=xw0_sb, in_=xw0[:, :].bitcast(F32R))
            xT_sb = xw0_sb[:, 0:N]
            w0_sb = {
                "q": xw0_sb[:, N:N + HID],
                "k": xw0_sb[:, N + HID:N + 2 * HID],
                "v": xw0_sb[:, N + 2 * HID:N + 3 * HID],
            }
            bias_all = const.tile([128, 16], F32, tag="bias_all")
            dma(out=bias_all, in_=ballp[:, :])
            bias_sb = {
                "b0q": bias_all[:, 0:2], "b1q": bias_all[:, 2:4],
                "b2q": bias_all[:, 4:6], "b0k": bias_all[:, 6:8],
                "b1k": bias_all[:, 8:10], "b2k": bias_all[:, 10:11],
                "b0v": bias_all[:, 11:13], "b1v": bias_all[:, 13:15],
                "bvp": bias_all[0:NH, 15:16],
            }
            w1_sb = {}
            for nm in ("q", "k", "v"):
                w1_sb[nm] = const.tile([128, 2, HID], F32R, tag=f"w1{nm}_sb", name=f"w1{nm}_sb")
            w2q_sb = const.tile([128, 2, 256], F32R, tag="w2q_sb")
            w2k_sb = const.tile([128, 2, HF], F32R, tag="w2k_sb")
            w2vp_sb = const.tile([128, 2, NH], F32R, tag="w2vp_sb")
            for nm, t in (("q", w1q), ("k", w1k)):
                for kc in range(2):
                    dma(out=w1_sb[nm][:, kc, :], in_=t[kc * 128:(kc + 1) * 128, :].bitcast(F32R))
            for kc in range(2):
                dma(out=w2q_sb[:, kc, :], in_=w2q[kc * 128:(kc + 1) * 128, :].bitcast(F32R))
                dma(out=w2k_sb[:, kc, :], in_=w2k[kc * 128:(kc + 1) * 128, :].bitcast(F32R))
            omi_sb = const.tile([128, 2, 128], F32R, tag="omi_sb")
            dma(out=omi_sb, in_=omi[:, :, :].bitcast(F32R))
            for kc in range(2):
                dma(out=w1_sb["v"][:, kc, :], in_=w1v[kc * 128:(kc + 1) * 128, :].bitcast(F32R))
                dma(out=w2vp_sb[:, kc, :], in_=w2vp[kc * 128:(kc + 1) * 128, :].bitcast(F32R))
            vpT = const.tile([128, 8, 2, NH], F32R, tag="vpT")
            dma(out=vpT[:, :, 0, :],
                in_=onesc[:, :].rearrange("p (a b) -> p a b", a=8).bitcast(F32R))
            ident_sb = const.tile([128, 128], F32, tag="ident_sb")
            dma(out=ident_sb, in_=ident[:, :])

            # warm the exp/tanh ACT table set at t=0 (table load ~2.7us)
            warm = const.tile([1, 1], F32, tag="warm")
            nc.vector.memset(warm, 0.0)
            warm2 = const.tile([1, 1], F32, tag="warm2")
            nc.scalar.activation(warm2, warm, AF.Exp)

            # ---- MLPs (transposed activations: [features, tokens]) ----
            def hidden_layer(w_sb, in_sb, in_parts, b_sb, out_name):
                """out = tanh(w^T @ in + b), out [128,2,1024]."""
                out_sb = acts.tile([128, 2, N], F32R, tag=out_name, name=out_name)
                for pt in range(2):
                    ps = pp2.tile([128, N], F32, tag="ps2", name="ps_mlp")
                    for qc2 in range(2):
                        cs = slice(qc2 * 512, qc2 * 512 + 512)
                        if in_parts == D:  # layer 0: K = D = 8
                            nc.tensor.matmul(
                                ps[:, cs],
                                lhsT=w_sb[:, pt * 128:(pt + 1) * 128],
                                rhs=in_sb[:, cs],
                            )
                        else:  # K = 256 in two chunks
                            for kc in range(2):
                                nc.tensor.matmul(
                                    ps[:, cs],
                                    lhsT=w_sb[:, kc, pt * 128:(pt + 1) * 128],
                                    rhs=in_sb[:, kc, cs],
                                    start=(kc == 0), stop=(kc == 1),
                                )
                    nc.scalar.activation(out_sb[:, pt, :], ps, AF.Tanh,
                                         bias=b_sb[:, pt:pt + 1])
                return out_sb

            a0q = hidden_layer(w0_sb["q"], xT_sb, D, bias_sb["b0q"], "a0q")
            a1q = hidden_layer(w1_sb["q"], a0q, HID, bias_sb["b1q"], "a1q")
            a0k = hidden_layer(w0_sb["k"], xT_sb, D, bias_sb["b0k"], "a0k")
            a1k = hidden_layer(w1_sb["k"], a0k, HID, bias_sb["b1k"], "a1k")
            a0v = hidden_layer(w0_sb["v"], xT_sb, D, bias_sb["b0v"], "a0v")
            a1v = hidden_layer(w1_sb["v"], a0v, HID, bias_sb["b1v"], "a1v")

            # Q_d^T [128, 2(d), 1024]
            QdT = acts.tile([128, 2, N], F32R, tag="QdT")
            for d in range(2):
                for qc2 in range(2):
                    cs = slice(qc2 * 512, qc2 * 512 + 512)
                    ps = pA.tile([128, 512], F32, tag="psA", name="ps_l2")
                    for kc in range(2):
                        nc.tensor.matmul(
                            ps,
                            lhsT=w2q_sb[:, kc, d * 128:(d + 1) * 128],
                            rhs=a1q[:, kc, cs],
                            start=(kc == 0), stop=(kc == 1),
                        )
                    nc.vector.tensor_scalar_add(QdT[:, d, cs], ps,
                                                bias_sb["b2q"][:, d:d + 1])
            # K^T [128, 1024] (pre-scaled)
            KT = acts.tile([128, N], F32R, tag="KT")
            for qc2 in range(2):
                cs = slice(qc2 * 512, qc2 * 512 + 512)
                ps = pA.tile([128, 512], F32, tag="psA", name="ps_l2k")
                for kc in range(2):
                    nc.tensor.matmul(
                        ps,
                        lhsT=w2k_sb[:, kc, :],
                        rhs=a1k[:, kc, cs],
                        start=(kc == 0), stop=(kc == 1),
                    )
                nc.vector.tensor_scalar_add(KT[:, cs], ps, bias_sb["b2k"][:, 0:1])
            # vp [4, 1024] = per-head value.pW
            vp_sb = acts.tile([NH, N], F32, tag="vp_sb")
            for qc2 in range(2):
                cs = slice(qc2 * 512, qc2 * 512 + 512)
                ps = pA.tile([NH, 512], F32, tag="psA", name="ps_vp")
                for kc in range(2):
                    nc.tensor.matmul(
                        ps,
                        lhsT=w2vp_sb[:, kc, :],
                        rhs=a1v[:, kc, cs],
                        start=(kc == 0), stop=(kc == 1),
                    )
                nc.vector.tensor_scalar_add(vp_sb[:, cs], ps, bias_sb["bvp"][:, 0:1])

            # vpT [128, kt, (one|vp), h]: fill vp column per k-tile
            for kt in range(8):
                tp = pA.tile([128, NH], F32, tag="psA", name="tp_vp")
                nc.tensor.transpose(tp, in_=vp_sb[:, kt * 128:(kt + 1) * 128],
                                    identity=ident_sb[0:NH, 0:NH])
                nc.vector.tensor_copy(vpT[:, kt, 1, :], tp)

            # ---- attention ----
            Y = const.tile([128, 2, 2, 4], F32, tag="Y")  # [p, d, qc, qs]
            for d in range(2):
                for qc in range(2):
                    cs = slice(qc * 512, qc * 512 + 512)
                    # per-head [den;num] accumulators, one PSUM bank each
                    Ah = []
                    for hh in range(NH):
                        a = pA.tile([2, 512], F32, tag="psA", name=f"A{hh}")
                        Ah.append(a)
                    for kt in range(8):
                        for hp in range(2):
                            S = pp2.tile([128, 2, 512], F32, tag="ps2", name="S_sc")
                            for i in range(2):
                                hh = 2 * hp + i
                                nc.tensor.matmul(
                                    S[:, i, :],
                                    lhsT=KT[32 * hh:32 * hh + 32,
                                            kt * 128:(kt + 1) * 128],
                                    rhs=QdT[32 * hh:32 * hh + 32, d, cs],
                                    tile_position=(32 * hh, 0),
                                )
                            P = pexp.tile([128, 2, 512], F32R, tag="P", name="P_exp")
                            nc.scalar.activation(P[:, :, :], S[:, :, :], AF.Exp)
                            if kt // 4 == qc:
                                col = kt * 128 - qc * 512
                                nc.vector.tensor_mul(
                                    P[:, :, col:col + 128],
                                    P[:, :, col:col + 128], omi_sb)
                            for i in range(2):
                                hh = 2 * hp + i
                                nc.tensor.matmul(
                                    Ah[hh][:, :],
                                    lhsT=vpT[:, kt, :, hh],
                                    rhs=P[:, i, :],
                                    start=(kt == 0), stop=(kt == 7),
                                )
                    # epilogue: gather per-head [den;num] rows (32-aligned),
                    # transpose, divide
                    A_sb = asb.tile([128, 512], F32, tag="A_sb", name="A_sb")
                    for hh in range(NH):
                        nc.vector.tensor_copy(A_sb[32 * hh:32 * hh + 2, :],
                                              Ah[hh][:, :])
                    At = pA.tile([128, 4, 128], F32, tag="psA", name="At")
                    for qs in range(4):
                        nc.tensor.transpose(At[:, qs, :],
                                            in_=A_sb[:, qs * 128:(qs + 1) * 128],
                                            identity=ident_sb)
                    rt = small.tile([128, 4, NH], F32, tag="rt", name="rt")
                    nc.vector.reciprocal(rt, At[:, :, 0:97:32])
                    yh = small.tile([128, 4, NH], F32, tag="yh", name="yh")
                    nc.vector.tensor_mul(yh, rt, At[:, :, 1:98:32])
                    t2 = small.tile([128, 4], F32, tag="t2", name="t2")
                    nc.vector.tensor_add(t2, yh[:, :, 0], yh[:, :, 1])
                    nc.vector.tensor_add(Y[:, d, qc, :], yh[:, :, 2], yh[:, :, 3])
                    nc.vector.tensor_add(Y[:, d, qc, :], Y[:, d, qc, :], t2)

            dma(out=yout[:, :], in_=Y[:, :, :, :])

    if not nc.is_finalized():
        nc.finalize()
    return nc


def _host_prep(inputs):
    m0, m1, m2 = _made_masks()
    f = np.float32
    qW0m = (np.asarray(inputs["qW0"], f) * m0)
    qW1m = (np.asarray(inputs["qW1"], f) * m1)
    qW2m = (np.asarray(inputs["qW2"], f) * m2)
    s = np.float32(DH ** -0.5)
    kW2s = np.asarray(inputs["kW2"], f) * s
    kb2s = np.asarray(inputs["kb2"], f) * s
    pw = np.asarray(inputs["pW"], f)[:, 0]
    vpW = (np.asarray(inputs["vW2"], f) * pw).reshape(HID, NH, DH).sum(-1)
    vpb = (np.asarray(inputs["vb2"], f) * pw).reshape(NH, DH).sum(-1)

    def col2(v):  # [256] -> [128, 2]
        return np.ascontiguousarray(np.asarray(v, f).reshape(2, 128).T)

    ballp = np.zeros((128, 16), f)

    w0pack = np.concatenate(
        [qW0m, np.asarray(inputs["kW0"], f), np.asarray(inputs["vW0"], f)],
        axis=1)  # (8, 768)
    shared = {
        "w1q": np.ascontiguousarray(qW1m),
        "w1k": np.ascontiguousarray(np.asarray(inputs["kW1"], f)),
        "w1v": np.ascontiguousarray(np.asarray(inputs["vW1"], f)),
        "w2k": np.ascontiguousarray(kW2s),
        "w2vp": np.ascontiguousarray(vpW.astype(f)),
        "ident": np.eye(128, dtype=f),
        "onesc": np.ones((128, 32), f),
        "omi": np.ascontiguousarray(np.broadcast_to(
            (1.0 - np.eye(128, dtype=f))[:, None, :], (128, 2, 128)).copy()),
    }
    ballp[:, 0:2] = col2(inputs["qb0"])
    ballp[:, 2:4] = col2(inputs["qb1"])
    ballp[:, 6:8] = col2(inputs["kb0"])
    ballp[:, 8:10] = col2(inputs["kb1"])
    ballp[:, 10] = kb2s
    ballp[:, 11:13] = col2(inputs["vb0"])
    ballp[:, 13:15] = col2(inputs["vb1"])
    ballp[0:NH, 15] = vpb.astype(f)
    x = np.asarray(inputs["x"], f)
    qb2 = np.asarray(inputs["qb2"], f)
    in_maps = []
    for c in range(N_CORES):
        b = c // 4
        d0, d1 = 2 * (c % 4), 2 * (c % 4) + 1
        m = dict(shared)
        m["xw0"] = np.ascontiguousarray(
            np.concatenate([x[b].T, w0pack], axis=1))
        m["w2q"] = np.ascontiguousarray(
            np.concatenate([qW2m[:, d0::D], qW2m[:, d1::D]], axis=1))
        bp = ballp.copy()
        bp[:, 4:6] = np.stack([qb2[d0::D], qb2[d1::D]], axis=1)
        m["ballp"] = bp
        in_maps.append(m)
    return in_maps


def kernel(**inputs):
    global LAST_RESULT
    if "nc" not in _prog_cache:
        _prog_cache["nc"] = _build_program()
    nc = _prog_cache["nc"]
    in_maps = _host_prep(inputs)
    res = run_bass_kernel_spmd(nc, in_maps, core_ids=list(range(N_CORES)),
                               **RUN_KWARGS)
    LAST_RESULT = res
    x = np.asarray(inputs["x"], np.float32)
    pb = np.asarray(inputs["pb"], np.float32)
    y = np.zeros((B, N, D), np.float32)
    for c in range(N_CORES):
        r = np.asarray(res.results[c]["yout"]).reshape(128, 2, 2, 4)
        b = c // 4
        for d in range(2):
            y[b, :, 2 * (c % 4) + d] = r[:, d].transpose(1, 2, 0).reshape(N)
    y += pb[0]
    return y, np.zeros_like(x)
